# revision 26
# baseline (speedup 1.0000x reference)
"""3-layer GAT on 8 Trainium2 NeuronCores.

Strategy (graph/data parallel, per sharding hint):
 - Nodes sharded 2500/core (8 cores). Edges partitioned by dst shard and
   sorted by dst; each core owns the edges targeting its nodes.
 - Per layer: node phase computes feat = h @ W (plus residual and the
   attention projections el/er folded into the same matmul as extra
   columns), packs [feat|1|el] rows into a bf16 table shard, AllGather
   builds the full 20000-row table in every core's HBM.
 - Edge phase: dma_gather pulls per-edge src rows; e = leakyrelu(el_src +
   er_dst) with er_dst broadcast via a one-hot-transpose matmul; exp on
   ScalarE; the softmax numerator AND denominator are accumulated with
   one-hot matmuls on the tensor engine (segment-sum), normalizing after
   aggregation (exp-max subtraction is unnecessary: |e| <= ~6).
 - Layer 2 additionally re-broadcasts 1/denom per edge to emit alpha.

Host side: shard/sort/pad edges (8.8% pad), build gather indices,
one-hot-transpose tables, pack weights. Device outputs per-core logits,
prior, alpha; host reassembles full outputs.
"""
import os
import numpy as np

import concourse.bacc as bacc
import concourse.bass as bass
import concourse.mybir as mybir
import concourse.tile as tile
from concourse.bass_utils import run_bass_kernel_spmd

dt = mybir.dt
F32 = dt.float32
BF16 = dt.bfloat16
NP_BF16 = np.dtype(dt.np(BF16))

# Problem constants (nn_GAT_student_11003706212773)
N, E, DIN = 20000, 320000, 512
F, H, C = 128, 4, 64
NEG_SLOPE = 0.2

NCORES = 8
NCN = N // NCORES          # nodes per core = 2500
GRP = 125                  # nodes per psum group
NGRP = NCN // GRP          # 20 groups per core
NCHUNK = 17                # edge chunks (of 128) per group, uniform across cores
EPG = NCHUNK * 128         # padded edges per group = 2176
EPC = NGRP * EPG           # padded edges per core = 43520
NCOL = NGRP * NCHUNK       # chunk columns per core = 340

# Per-layer config: (Kc, Wf, Wr, HH, Fl, HB, ROW, EL32, act)
#   Kc: 128-contraction chunks; Wf: feat cols; Wr: res cols; HH: heads;
#   Fl: feat/head; HB: head block = Fl+1; ROW: bf16 row len; EL32: f32 idx of el
LAYERS = [
    dict(Kc=4, Wf=128, Wr=0, HH=1, Fl=128, HB=129, ROW=256, EL32=96, act=True),
    dict(Kc=1, Wf=512, Wr=512, HH=4, Fl=128, HB=129, ROW=640, EL32=258, act=True),
    dict(Kc=4, Wf=256, Wr=64, HH=4, Fl=64, HB=65, ROW=384, EL32=130, act=False),
]


# ----------------------------------------------------------------------------
# Host-side prep
# ----------------------------------------------------------------------------

def _prep_graph(src, dst):
    """Partition+sort+pad edges per core; build device index arrays."""
    src = np.asarray(src).astype(np.int64)
    dst = np.asarray(dst).astype(np.int64)
    cores = []
    for c in range(NCORES):
        ids = np.where(dst // NCN == c)[0]
        l = dst[ids] - c * NCN
        order = np.argsort(l, kind="stable")
        ids, l = ids[order], l[order]
        g = l // GRP
        off = l - g * GRP

        src_arr = np.zeros((NGRP, EPG), np.int64)
        off_arr = np.full((NGRP, EPG), -1.0, np.float32)
        em_arr = np.full((NGRP, EPG), -1, np.int64)
        ohT = np.zeros((GRP, NCOL, 128), np.float32)
        for gi in range(NGRP):
            m = g == gi
            cnt = int(m.sum())
            assert cnt <= EPG
            src_arr[gi, :cnt] = src[ids[m]]
            off_arr[gi, :cnt] = off[m]
            em_arr[gi, :cnt] = ids[m]
            sl = np.arange(cnt)
            ohT[off[m], gi * NCHUNK + sl // 128, sl % 128] = 1.0

        # gather idx: slot i -> [i%16, i//16], per group, replicated x8
        idx16 = np.concatenate(
            [src_arr[gi].reshape(EPG // 16, 16).T for gi in range(NGRP)], axis=1
        ).astype(np.int16)
        idx_in = np.tile(idx16, (8, 1)).copy()
        # partition-major per-edge arrays: slot i -> [i%128, g*NCHUNK + i//128]
        def pmaj(a):
            return np.ascontiguousarray(
                a.reshape(NGRP, NCHUNK, 128).transpose(2, 0, 1).reshape(128, NCOL)
            )
        cores.append(dict(
            idx_in=idx_in,
            off_in=pmaj(off_arr).astype(NP_BF16),
            ohT_in=ohT.astype(np.dtype(dt.np(dt.float8e4))),
            em=pmaj(em_arr.astype(np.float64)).astype(np.int64),
        ))
    return cores


def _prep_weights(W0, al0, ar0, b0, W1, al1, ar1, b1, resW1, W2, al2, ar2, b2, resW2):
    def pack(cols):
        w = np.concatenate(cols, axis=1).astype(np.float32)  # [Din, TOT]
        Din = w.shape[0]
        return np.ascontiguousarray(
            w.reshape(Din // 128, 128, w.shape[1]).transpose(1, 0, 2))

    wal0 = W0 @ al0[0, 0]                      # [512]
    war0 = W0 @ ar0[0, 0]
    w0 = pack([W0, wal0[:, None], war0[:, None]])          # [128, 4, 130]

    wal1 = np.stack([W1[:, h * F:(h + 1) * F] @ al1[0, h] for h in range(H)], 1)
    war1 = np.stack([W1[:, h * F:(h + 1) * F] @ ar1[0, h] for h in range(H)], 1)
    w1 = pack([W1, resW1, wal1, war1])                     # [128, 1, 1032]

    wal2 = np.stack([W2[:, h * C:(h + 1) * C] @ al2[0, h] for h in range(H)], 1)
    war2 = np.stack([W2[:, h * C:(h + 1) * C] @ ar2[0, h] for h in range(H)], 1)
    resW2m = resW2.reshape(DIN, H, C).mean(axis=1)         # [512, 64]
    w2 = pack([W2, resW2m, wal2, war2])                    # [128, 4, 328]

    b0r = np.tile(b0[None, :], (128, 1)).astype(np.float32)
    b1r = np.tile(b1[None, :], (128, 1)).astype(np.float32)
    b2r = np.tile(b2.reshape(H, C).mean(0)[None, :], (128, 1)).astype(np.float32)
    return w0, w1, w2, b0r, b1r, b2r


# ----------------------------------------------------------------------------
# Device kernel
# ----------------------------------------------------------------------------

def _build_kernel():
    nc = bacc.Bacc("TRN2", target_bir_lowering=False, debug=False,
                   num_devices=NCORES)

    xT_in = nc.dram_tensor("xT_in", [128, 4, NCN], F32, kind="ExternalInput")
    w0_in = nc.dram_tensor("w0_in", [128, 4, 130], F32, kind="ExternalInput")
    w1_in = nc.dram_tensor("w1_in", [128, 1, 1032], F32, kind="ExternalInput")
    w2_in = nc.dram_tensor("w2_in", [128, 4, 328], F32, kind="ExternalInput")
    b0_in = nc.dram_tensor("b0_in", [128, 128], F32, kind="ExternalInput")
    b1_in = nc.dram_tensor("b1_in", [128, 512], F32, kind="ExternalInput")
    b2_in = nc.dram_tensor("b2_in", [128, 64], F32, kind="ExternalInput")
    idx_in = nc.dram_tensor("idx_in", [128, NGRP * EPG // 16], dt.int16, kind="ExternalInput")
    off_in = nc.dram_tensor("off_in", [128, NCOL], BF16, kind="ExternalInput")
    ohT_in = nc.dram_tensor("ohT_in", [GRP, NCOL, 128], dt.float8e4, kind="ExternalInput")

    logits_out = nc.dram_tensor("logits_out", [NCN, C], F32, kind="ExternalOutput")
    prior_out = nc.dram_tensor("prior_out", [NGRP, GRP], F32, kind="ExternalOutput")
    alpha_out = nc.dram_tensor("alpha_out", [128, NCOL, H], F32, kind="ExternalOutput")

    w_ins = [w0_in, w1_in, w2_in]
    b_ins = [b0_in, b1_in, b2_in]

    with tile.TileContext(nc) as tc:
        with (
            tc.tile_pool(name="const", bufs=1) as cp,
            tc.tile_pool(name="state", bufs=1) as st,
            tc.tile_pool(name="dram", bufs=1, space="DRAM") as dp,
        ):
            # ---- constants
            w_t = [cp.tile(list(w.shape), F32, name=f"w{i}_t")
                   for i, w in enumerate(w_ins)]
            b_t = [cp.tile(list(b.shape), F32, name=f"b{i}_t")
                   for i, b in enumerate(b_ins)]
            for s, d in zip(w_ins + b_ins, w_t + b_t):
                nc.sync.dma_start(d[:], s[:])
            idx_sb = cp.tile([128, NGRP * EPG // 16], dt.int16, name="idx_sb")
            nc.sync.dma_start(idx_sb[:], idx_in[:])
            off_sb = cp.tile([128, NCOL], BF16, name="off_sb")
            nc.sync.dma_start(off_sb[:], off_in[:])

            iota_i = cp.tile([128, 128], dt.int32, name="iota_i")
            nc.gpsimd.iota(iota_i[:], pattern=[[1, 128]], base=0, channel_multiplier=0)
            iota_bf = cp.tile([128, 128], BF16, name="iota_bf")
            nc.vector.tensor_copy(iota_bf[:], iota_i[:])
            iotap_i = cp.tile([128, 1], dt.int32, name="iotap_i")
            nc.gpsimd.iota(iotap_i[:], pattern=[[1, 1]], base=0, channel_multiplier=1)
            iotap_f = cp.tile([128, 1], F32, name="iotap_f")
            nc.vector.tensor_copy(iotap_f[:], iotap_i[:])
            iota_f = cp.tile([128, 128], F32, name="iota_f")
            nc.vector.tensor_copy(iota_f[:], iota_i[:])
            zbias = cp.tile([128, 1], F32, name="zbias")
            nc.vector.memset(zbias[:], 0.0)
            ident = cp.tile([128, 128], F32, name="ident")
            nc.vector.tensor_scalar(ident[:], iota_f[:], iotap_f[:], None,
                                    mybir.AluOpType.is_equal)

            # ---- state
            h1T = st.tile([128, NGRP, GRP], F32, name="h1T")
            h2T = st.tile([128, 4, NGRP, GRP], F32, name="h2T")
            er_sb = st.tile([GRP, NGRP, 8], BF16, name="er_sb")
            res2_sb = st.tile([GRP, NGRP, C], F32, name="res2_sb")
            prior_sb = st.tile([GRP, NGRP], F32, name="prior_sb")

            # ---- DRAM scratch
            ag_in = [dp.tile([NCN, L["ROW"]], BF16, name=f"ag{i}")
                     for i, L in enumerate(LAYERS)]
            REPS_ = int(os.environ.get("GAT_REPS", "1"))
            tables_r = [[dp.tile([N, L["ROW"]], BF16, addr_space="Shared",
                                 name=f"table{i}_r{r}")
                         for i, L in enumerate(LAYERS)]
                        for r in range(REPS_)]
            res1_d = dp.tile([NCN, 512], F32, name="res1_d")

            NLAYERS = int(os.environ.get("GAT_LAYERS", "3"))
            REPS = int(os.environ.get("GAT_REPS", "1"))
            DO_AG = os.environ.get("GAT_AG", "1") == "1"
            DO_EDGE = os.environ.get("GAT_EDGE", "1") == "1"
            ESTAGE = int(os.environ.get("GAT_ESTAGE", "9"))
            for rep in range(REPS):
             tables = tables_r[rep]
             for li, L in enumerate(LAYERS[:NLAYERS]):
                Kc, Wf, Wr, HH, Fl, HB = (L["Kc"], L["Wf"], L["Wr"], L["HH"],
                                          L["Fl"], L["HB"])
                ROW, EL32, act = L["ROW"], L["EL32"], L["act"]
                wt = w_t[li]

                # ============================ node phase =====================
                with (
                    tc.tile_pool(name=f"np{li}", bufs=2) as npool,
                    tc.tile_pool(name=f"npp{li}", bufs=2, space="PSUM") as npp,
                ):
                    for g in range(NGRP):
                        # lhsT source
                        if li == 0:
                            hT_t = npool.tile([128, Kc, GRP], F32, name="hT_t")
                            nc.sync.dma_start(hT_t[:], xT_in[:, :, g * GRP:(g + 1) * GRP])
                            def lhs(k):
                                return hT_t[:, k, :]
                        elif li == 1:
                            def lhs(k):
                                return h1T[:, g, :]
                        else:
                            def lhs(k):
                                return h2T[:, k, g, :]

                        TOT = Wf + Wr + 2 * HH
                        if TOT <= 512:
                            # one fused matmul: [feat|res|elr]
                            all_ps = npp.tile([GRP, TOT], F32, name="feat_ps")
                            for k in range(Kc):
                                nc.tensor.matmul(all_ps[:], lhs(k), wt[:, k, 0:TOT],
                                                 start=(k == 0), stop=(k == Kc - 1))
                            feat_ps = all_ps[:, 0:Wf]
                            res_ps = all_ps[:, Wf:Wf + Wr] if Wr else None
                            elr_ps = all_ps[:, Wf + Wr:TOT]
                        else:
                            feat_tp = npp.tile([GRP, Wf], F32, name="feat_ps")
                            elr_tp = npp.tile([GRP, 2 * HH], F32, name="elr_ps")
                            res_tp = (npp.tile([GRP, Wr], F32, name="res_ps")
                                      if Wr else None)
                            for k in range(Kc):
                                nc.tensor.matmul(feat_tp[:], lhs(k), wt[:, k, 0:Wf],
                                                 start=(k == 0), stop=(k == Kc - 1))
                                if Wr:
                                    nc.tensor.matmul(res_tp[:], lhs(k),
                                                     wt[:, k, Wf:Wf + Wr],
                                                     start=(k == 0), stop=(k == Kc - 1))
                                nc.tensor.matmul(elr_tp[:], lhs(k),
                                                 wt[:, k, Wf + Wr:Wf + Wr + 2 * HH],
                                                 start=(k == 0), stop=(k == Kc - 1))
                            feat_ps = feat_tp[:]
                            res_ps = res_tp[:] if Wr else None
                            elr_ps = elr_tp[:]

                        # pack row = [feat|1]*HH + el
                        row_t = npool.tile([GRP, ROW], BF16, name="row_t")
                        nc.vector.memset(row_t[:], 0.0)
                        rhead = row_t[:, 0:HH * HB].rearrange(
                            "p (h b) -> p h b", b=HB)
                        nc.vector.tensor_copy(
                            rhead[:, :, 0:Fl],
                            feat_ps.rearrange("p (h f) -> p h f", f=Fl))
                        nc.vector.memset(rhead[:, :, Fl:Fl + 1], 1.0)
                        nc.vector.tensor_copy(
                            row_t[:, 2 * EL32:2 * EL32 + 2 * HH].bitcast(F32),
                            elr_ps[:, 0:HH])
                        # er hi/lo split (bf16 pair for exact fp32 rebuild)
                        nc.vector.tensor_copy(er_sb[:, g, 0:HH], elr_ps[:, HH:2 * HH])
                        hi_f = npool.tile([GRP, HH], F32, name="hi_f")
                        nc.vector.tensor_copy(hi_f[:], er_sb[:, g, 0:HH])
                        nc.vector.tensor_tensor(er_sb[:, g, HH:2 * HH],
                                                elr_ps[:, HH:2 * HH], hi_f[:],
                                                mybir.AluOpType.subtract)
                        # residual stash
                        if li == 1:
                            res_t = npool.tile([GRP, 512], F32, name="res_t")
                            nc.vector.tensor_copy(res_t[:], res_ps)
                            nc.sync.dma_start(res1_d[g * GRP:(g + 1) * GRP, :], res_t[:])
                        elif li == 2:
                            nc.vector.tensor_copy(res2_sb[:, g, :], res_ps[:, 0:C])
                        nc.sync.dma_start(ag_in[li][g * GRP:(g + 1) * GRP, :], row_t[:])

                # ============================ all-gather =====================
                if DO_AG:
                    nc.gpsimd.collective_compute(
                        "AllGather", mybir.AluOpType.bypass,
                        replica_groups=[list(range(NCORES))],
                        ins=[ag_in[li].opt()], outs=[tables[li].opt()],
                    )
                if not DO_EDGE:
                    continue

                # ============================ edge phase =====================
                with (
                    tc.tile_pool(name=f"eg{li}", bufs=3) as gp,
                    tc.tile_pool(name=f"eo{li}", bufs=2) as op,
                    tc.tile_pool(name=f"es{li}", bufs=2) as sp_pool,
                    tc.tile_pool(name=f"ew{li}", bufs=2) as wp,
                    tc.tile_pool(name=f"epn{li}", bufs=(2 if HH == 1 else 1), space="PSUM") as pn,
                    tc.tile_pool(name=f"epe{li}", bufs=2, space="PSUM") as pe,
                ):
                    if li == 2:
                        exbf_sb = wp.tile([128, NCOL, H], BF16, name="exbf_sb",
                                          bufs=1)
                    for g in range(int(os.environ.get('GAT_NGRP', NGRP))):
                        grow_t = gp.tile([128, NCHUNK, ROW], BF16, name="grow")
                        nc.gpsimd.dma_gather(
                            grow_t[:], tables[li][:],
                            idx_sb[:, g * (EPG // 16):(g + 1) * (EPG // 16)],
                            num_idxs=EPG, num_idxs_reg=EPG, elem_size=ROW,
                            single_packet=False,
                        )
                        grow = grow_t[:]
                        ohT_t = op.tile([GRP, NCHUNK, 128], dt.float8e4, name="ohT_t")
                        nc.sync.dma_start(
                            ohT_t[:], ohT_in[:, g * NCHUNK:(g + 1) * NCHUNK, :])
                        ohT = ohT_t[:]
                        if li == 1:
                            res_rd = op.tile([GRP, 512], F32, name="res_rd")
                            nc.sync.dma_start(res_rd[:], res1_d[g * GRP:(g + 1) * GRP, :])

                        if ESTAGE <= 1:
                            continue
                        # er_dst via one-hot-transpose matmul (hi+lo)
                        er_ps = pe.tile([128, NCHUNK * 8], F32, name="er_ps")
                        for ck in range(NCHUNK):
                            nc.tensor.matmul(er_ps[:, ck * 8:ck * 8 + 2 * HH],
                                             ohT[:, ck, :], er_sb[:, g, 0:2 * HH],
                                             start=True, stop=True)

                        if ESTAGE <= 2:
                            continue
                        # e = lrelu(el_src + er_hi + er_lo); ex = exp(e)
                        erv = er_ps[:].rearrange("p (c x) -> p c x", x=8)
                        elv = grow.bitcast(F32)[:, :, EL32:EL32 + HH]
                        e_f = wp.tile([128, NCHUNK, HH], F32, name="e_f")
                        nc.vector.tensor_tensor(e_f[:], elv, erv[:, :, 0:HH],
                                                mybir.AluOpType.add)
                        nc.vector.tensor_tensor(e_f[:], e_f[:], erv[:, :, HH:2 * HH],
                                                mybir.AluOpType.add)
                        lr_t = wp.tile([128, NCHUNK, HH], F32, name="lr_t")
                        nc.vector.tensor_scalar(lr_t[:], e_f[:], NEG_SLOPE, None,
                                                mybir.AluOpType.mult)
                        nc.vector.tensor_tensor(e_f[:], e_f[:], lr_t[:],
                                                mybir.AluOpType.max)
                        ex_f = wp.tile([128, NCHUNK, HH], F32, name="ex_f")
                        nc.scalar.activation(ex_f[:], e_f[:],
                                             mybir.ActivationFunctionType.Exp,
                                             bias=zbias[:])
                        if li == 2:
                            exbf = exbf_sb[:, g * NCHUNK:(g + 1) * NCHUNK, :]
                        else:
                            exbf_t = wp.tile([128, NCHUNK, HH], BF16, name="exbf_t")
                            exbf = exbf_t[:]
                        nc.vector.tensor_copy(exbf, ex_f[:])

                        if ESTAGE <= 3:
                            continue
                        # forward one-hot scaled by ex
                        oh_f = op.tile([128, NCHUNK, 128], BF16, name="oh_f")
                        nc.vector.tensor_tensor(
                            oh_f[:],
                            iota_bf[:].unsqueeze(1).broadcast_to([128, NCHUNK, 128]),
                            off_sb[:, g * NCHUNK:(g + 1) * NCHUNK]
                                .unsqueeze(2).broadcast_to([128, NCHUNK, 128]),
                            mybir.AluOpType.is_equal)
                        sp_t = sp_pool.tile([128, NCHUNK, HH, 128], BF16, name="sp_t")
                        nc.vector.tensor_tensor(
                            sp_t[:],
                            oh_f[:].unsqueeze(2).broadcast_to([128, NCHUNK, HH, 128]),
                            (exbf if li == 2 else exbf_t[:])
                                .unsqueeze(3).broadcast_to([128, NCHUNK, HH, 128]),
                            mybir.AluOpType.mult)

                        if ESTAGE <= 4:
                            continue
                        # segment-sum matmuls
                        num_ps = [pn.tile([128, HB], F32, name=f"num{h}")
                                  for h in range(HH)]
                        for ck in range(NCHUNK):
                            for h in range(HH):
                                nc.tensor.matmul(
                                    num_ps[h][:], sp_t[:, ck, h, :],
                                    grow[:, ck, h * HB:(h + 1) * HB],
                                    start=(ck == 0), stop=(ck == NCHUNK - 1))

                        if ESTAGE <= 5:
                            continue
                        # ---- eviction
                        recips = []
                        for h in range(HH):
                            den_t = wp.tile([128, 1], F32, name=f"den{h}")
                            nc.vector.tensor_scalar(den_t[:], num_ps[h][:, Fl:Fl + 1],
                                                    1e-9, None, mybir.AluOpType.max)
                            rec_t = wp.tile([128, 1], F32, name=f"rec{h}")
                            nc.vector.reciprocal(rec_t[:], den_t[:])
                            recips.append(rec_t)

                        if li < 2:
                            rst = wp.tile([GRP, HH * Fl], F32, name="rst")
                            rv = rst[:].rearrange("p (h f) -> p h f", f=Fl)
                            for h in range(HH):
                                nc.vector.tensor_scalar(
                                    rv[:, h, :], num_ps[h][:GRP, 0:Fl],
                                    recips[h][:GRP], None, mybir.AluOpType.mult)
                            if li == 1:
                                nc.vector.tensor_tensor(rst[:], rst[:], res_rd[:],
                                                        mybir.AluOpType.add)
                            nc.vector.tensor_tensor(rst[:], rst[:],
                                                    b_t[li][:GRP, 0:HH * Fl],
                                                    mybir.AluOpType.add)
                            # elu = max(x,0) - 1 + exp(min(x,0))
                            m_t = wp.tile([GRP, HH * Fl], F32, name="m_t")
                            nc.vector.tensor_scalar(m_t[:], rst[:], 0.0, None,
                                                    mybir.AluOpType.min)
                            em_t = wp.tile([GRP, HH * Fl], F32, name="em_t")
                            nc.scalar.activation(em_t[:], m_t[:],
                                                 mybir.ActivationFunctionType.Exp,
                                                 bias=zbias[:GRP])
                            nc.vector.tensor_scalar(rst[:], rst[:], 0.0, -1.0,
                                                    mybir.AluOpType.max,
                                                    mybir.AluOpType.add)
                            nc.vector.tensor_tensor(rst[:], rst[:], em_t[:],
                                                    mybir.AluOpType.add)
                            if li == 1:
                                pr_t = wp.tile([GRP, 1], F32, name="pr_t")
                                nc.vector.tensor_reduce(pr_t[:], rst[:],
                                                        mybir.AxisListType.X,
                                                        mybir.AluOpType.add)
                                nc.vector.tensor_scalar(prior_sb[:, g:g + 1], pr_t[:],
                                                        1.0 / 512, None,
                                                        mybir.AluOpType.mult)
                            # transpose h -> hT
                            for fc in range(HH * Fl // 128):
                                tp_ps = pe.tile([128, GRP], F32, name="tp_ps")
                                nc.tensor.transpose(
                                    tp_ps[:], rst[:, fc * 128:(fc + 1) * 128],
                                    ident[:GRP, :GRP])
                                if li == 0:
                                    nc.vector.tensor_copy(h1T[:, g, :], tp_ps[:])
                                else:
                                    nc.vector.tensor_copy(h2T[:, fc, g, :], tp_ps[:])
                        else:
                            # logits = 0.25*sum_h num_h/den_h + res2 + b2m
                            lg = wp.tile([GRP, C], F32, name="lg")
                            nc.vector.tensor_scalar(lg[:], num_ps[0][:GRP, 0:C],
                                                    recips[0][:GRP], 0.25,
                                                    mybir.AluOpType.mult,
                                                    mybir.AluOpType.mult)
                            for h in range(1, HH):
                                t_h = wp.tile([GRP, C], F32, name="t_h")
                                nc.vector.tensor_scalar(t_h[:], num_ps[h][:GRP, 0:C],
                                                        recips[h][:GRP], 0.25,
                                                        mybir.AluOpType.mult,
                                                        mybir.AluOpType.mult)
                                nc.vector.tensor_tensor(lg[:], lg[:], t_h[:],
                                                        mybir.AluOpType.add)
                            nc.vector.tensor_tensor(lg[:], lg[:], res2_sb[:, g, :],
                                                    mybir.AluOpType.add)
                            nc.vector.tensor_tensor(lg[:], lg[:], b_t[2][:GRP, :],
                                                    mybir.AluOpType.add)
                            nc.sync.dma_start(logits_out[g * GRP:(g + 1) * GRP, :], lg[:])

                            # alpha = ex * (1/den)[dst]
                            rec4 = wp.tile([GRP, HH], F32, name="rec4")
                            for h in range(HH):
                                nc.vector.tensor_copy(rec4[:, h:h + 1], recips[h][:GRP])
                            rec_hl = wp.tile([GRP, 8], BF16, name="rec_hl")
                            nc.vector.tensor_copy(rec_hl[:, 0:HH], rec4[:])
                            rhi_f = wp.tile([GRP, HH], F32, name="rhi_f")
                            nc.vector.tensor_copy(rhi_f[:], rec_hl[:, 0:HH])
                            nc.vector.tensor_tensor(rec_hl[:, 4:4 + HH], rec4[:],
                                                    rhi_f[:], mybir.AluOpType.subtract)
                            rd_ps = pe.tile([128, NCHUNK * 8], F32, name="rd_ps")
                            for ck in range(NCHUNK):
                                nc.tensor.matmul(rd_ps[:, ck * 8:ck * 8 + 8],
                                                 ohT[:, ck, :], rec_hl[:],
                                                 start=True, stop=True)
                            rdv = rd_ps[:].rearrange("p (c x) -> p c x", x=8)
                            radd = wp.tile([128, NCHUNK, HH], F32, name="radd")
                            nc.vector.tensor_copy(radd[:], rdv[:, :, 0:HH])
                            nc.vector.tensor_tensor(radd[:], radd[:],
                                                    rdv[:, :, 4:4 + HH],
                                                    mybir.AluOpType.add)
                            al_t = wp.tile([128, NCHUNK, HH], F32, name="al_t")
                            nc.vector.tensor_tensor(al_t[:], radd[:], exbf,
                                                    mybir.AluOpType.mult)
                            nc.sync.dma_start(
                                alpha_out[:, g * NCHUNK:(g + 1) * NCHUNK, :], al_t[:])

            # prior: [GRP, NGRP] -> transpose -> [NGRP, GRP] -> DMA
            if NLAYERS >= 2 and DO_EDGE:
              with tc.tile_pool(name="fin", bufs=1) as fp, \
                 tc.tile_pool(name="finp", bufs=1, space="PSUM") as fpp:
                pr_ps = fpp.tile([NGRP, GRP], F32, name="pr_ps")
                nc.tensor.transpose(pr_ps[:], prior_sb[:], ident[:GRP, :GRP])
                pr_sb = fp.tile([NGRP, GRP], F32, name="pr_sb")
                nc.vector.tensor_copy(pr_sb[:], pr_ps[:])
                nc.sync.dma_start(prior_out[:], pr_sb[:])


    nc.compile()
    return nc


_KERNEL_CACHE = {}


def kernel(x, src, dst, W0, al0, ar0, b0, W1, al1, ar1, b1, resW1,
           W2, al2, ar2, b2, resW2):
    x = np.asarray(x, np.float32)
    graph = _prep_graph(src, dst)
    w0, w1, w2, b0r, b1r, b2r = _prep_weights(
        np.asarray(W0), np.asarray(al0), np.asarray(ar0), np.asarray(b0),
        np.asarray(W1), np.asarray(al1), np.asarray(ar1), np.asarray(b1),
        np.asarray(resW1),
        np.asarray(W2), np.asarray(al2), np.asarray(ar2), np.asarray(b2),
        np.asarray(resW2))

    if "nc" not in _KERNEL_CACHE:
        _KERNEL_CACHE["nc"] = _build_kernel()
    nc = _KERNEL_CACHE["nc"]

    in_maps = []
    for c in range(NCORES):
        xc = x[c * NCN:(c + 1) * NCN]          # [2500, 512]
        xT = np.ascontiguousarray(
            xc.T.reshape(4, 128, NCN).transpose(1, 0, 2))
        g = graph[c]
        in_maps.append({
            "xT_in": xT, "w0_in": w0, "w1_in": w1, "w2_in": w2,
            "b0_in": b0r, "b1_in": b1r, "b2_in": b2r,
            "idx_in": g["idx_in"], "off_in": g["off_in"], "ohT_in": g["ohT_in"],
        })

    global _LAST_IN_MAPS
    _LAST_IN_MAPS = in_maps
    res = run_bass_kernel_spmd(nc, in_maps, core_ids=list(range(NCORES)))

    logits = np.concatenate([res.results[c]["logits_out"] for c in range(NCORES)], 0)
    prior = np.concatenate([res.results[c]["prior_out"].reshape(-1) for c in range(NCORES)], 0)
    att = np.zeros((E, H, 1), np.float32)
    for c in range(NCORES):
        em = graph[c]["em"]                    # [128, NCOL]
        al = res.results[c]["alpha_out"]       # [128, NCOL, H]
        valid = em >= 0
        att[em[valid], :, 0] = al[valid, :]
    return logits, prior, att


# revision 34
# speedup vs baseline: 84.3715x; 84.3715x over previous
"""3-layer GAT on 8 Trainium2 NeuronCores.

Strategy (graph/data parallel, per sharding hint):
 - Nodes sharded 2500/core (8 cores). Edges partitioned by dst shard and
   sorted by dst; each core owns the edges targeting its nodes.
 - Per layer: node phase computes feat = h @ W (plus residual and the
   attention projections el/er folded into the same matmul as extra
   columns), packs [feat|1|el] rows into a bf16 table shard, AllGather
   builds the full 20000-row table in every core's HBM.
 - Edge phase: dma_gather pulls per-edge src rows; e = leakyrelu(el_src +
   er_dst) with er_dst broadcast via a one-hot-transpose matmul; exp on
   ScalarE; the softmax numerator AND denominator are accumulated with
   one-hot matmuls on the tensor engine (segment-sum), normalizing after
   aggregation (exp-max subtraction is unnecessary: |e| <= ~6).
 - Layer 2 additionally re-broadcasts 1/denom per edge to emit alpha.

Host side: shard/sort/pad edges (8.8% pad), build gather indices,
one-hot-transpose tables, pack weights. Device outputs per-core logits,
prior, alpha; host reassembles full outputs.
"""
import os
import numpy as np

import concourse.bacc as bacc
import concourse.bass as bass
import concourse.mybir as mybir
import concourse.tile as tile
from concourse.bass_utils import run_bass_kernel_spmd

dt = mybir.dt
F32 = dt.float32
BF16 = dt.bfloat16
NP_BF16 = np.dtype(dt.np(BF16))

# Problem constants (nn_GAT_student_11003706212773)
N, E, DIN = 20000, 320000, 512
F, H, C = 128, 4, 64
NEG_SLOPE = 0.2

NCORES = 8
NCN = N // NCORES          # nodes per core = 2500
GRP = 125                  # nodes per psum group
NGRP = NCN // GRP          # 20 groups per core
NCHUNK = 17                # edge chunks (of 128) per group, uniform across cores
EPG = NCHUNK * 128         # padded edges per group = 2176
EPC = NGRP * EPG           # padded edges per core = 43520
NCOL = NGRP * NCHUNK       # chunk columns per core = 340

# Per-layer config: (Kc, Wf, Wr, HH, Fl, HB, ROW, EL32, act)
#   Kc: 128-contraction chunks; Wf: feat cols; Wr: res cols; HH: heads;
#   Fl: feat/head; HB: head block = Fl+1; ROW: bf16 row len; EL32: f32 idx of el
LAYERS = [
    dict(Kc=4, Wf=128, Wr=0, HH=1, Fl=128, HB=129, ROW=256, EL32=96, act=True),
    dict(Kc=1, Wf=512, Wr=512, HH=4, Fl=128, HB=129, ROW=640, EL32=258, act=True),
    dict(Kc=4, Wf=256, Wr=64, HH=4, Fl=64, HB=65, ROW=384, EL32=130, act=False),
]


# ----------------------------------------------------------------------------
# Host-side prep
# ----------------------------------------------------------------------------

def _prep_graph(src, dst):
    """Partition+sort+pad edges per core; build device index arrays."""
    src = np.asarray(src).astype(np.int64)
    dst = np.asarray(dst).astype(np.int64)
    cores = []
    for c in range(NCORES):
        ids = np.where(dst // NCN == c)[0]
        l = dst[ids] - c * NCN
        order = np.argsort(l, kind="stable")
        ids, l = ids[order], l[order]
        g = l // GRP
        off = l - g * GRP

        src_arr = np.zeros((NGRP, EPG), np.int64)
        off_arr = np.full((NGRP, EPG), -1.0, np.float32)
        em_arr = np.full((NGRP, EPG), -1, np.int64)
        ohT = np.zeros((GRP, NCOL, 128), np.float32)
        for gi in range(NGRP):
            m = g == gi
            cnt = int(m.sum())
            assert cnt <= EPG
            src_arr[gi, :cnt] = src[ids[m]]
            off_arr[gi, :cnt] = off[m]
            em_arr[gi, :cnt] = ids[m]
            sl = np.arange(cnt)
            ohT[off[m], gi * NCHUNK + sl // 128, sl % 128] = 1.0

        # gather idx: slot i -> [i%16, i//16], per group, replicated x8
        idx16 = np.concatenate(
            [src_arr[gi].reshape(EPG // 16, 16).T for gi in range(NGRP)], axis=1
        ).astype(np.int16)
        idx_in = np.tile(idx16, (8, 1)).copy()
        # partition-major per-edge arrays: slot i -> [i%128, g*NCHUNK + i//128]
        def pmaj(a):
            return np.ascontiguousarray(
                a.reshape(NGRP, NCHUNK, 128).transpose(2, 0, 1).reshape(128, NCOL)
            )
        cores.append(dict(
            idx_in=idx_in,
            off_in=pmaj(off_arr).astype(NP_BF16),
            ohT_in=ohT.astype(np.dtype(dt.np(dt.float8e4))),
            em=pmaj(em_arr.astype(np.float64)).astype(np.int64),
        ))
    return cores


def _prep_weights(W0, al0, ar0, b0, W1, al1, ar1, b1, resW1, W2, al2, ar2, b2, resW2):
    def pack(cols):
        w = np.concatenate(cols, axis=1).astype(np.float32)  # [Din, TOT]
        Din = w.shape[0]
        return np.ascontiguousarray(
            w.reshape(Din // 128, 128, w.shape[1]).transpose(1, 0, 2))

    wal0 = W0 @ al0[0, 0]                      # [512]
    war0 = W0 @ ar0[0, 0]
    w0 = pack([W0, wal0[:, None], war0[:, None]])          # [128, 4, 130]

    wal1 = np.stack([W1[:, h * F:(h + 1) * F] @ al1[0, h] for h in range(H)], 1)
    war1 = np.stack([W1[:, h * F:(h + 1) * F] @ ar1[0, h] for h in range(H)], 1)
    w1 = pack([W1, resW1, wal1, war1])                     # [128, 1, 1032]

    wal2 = np.stack([W2[:, h * C:(h + 1) * C] @ al2[0, h] for h in range(H)], 1)
    war2 = np.stack([W2[:, h * C:(h + 1) * C] @ ar2[0, h] for h in range(H)], 1)
    resW2m = resW2.reshape(DIN, H, C).mean(axis=1)         # [512, 64]
    w2 = pack([W2, resW2m, wal2, war2])                    # [128, 4, 328]

    b0r = np.tile(b0[None, :], (128, 1)).astype(np.float32)
    b1r = np.tile(b1[None, :], (128, 1)).astype(np.float32)
    b2r = np.tile(b2.reshape(H, C).mean(0)[None, :], (128, 1)).astype(np.float32)
    return w0, w1, w2, b0r, b1r, b2r


# ----------------------------------------------------------------------------
# Device kernel
# ----------------------------------------------------------------------------

def _build_kernel():
    nc = bacc.Bacc("TRN2", target_bir_lowering=False, debug=False,
                   num_devices=NCORES)

    xT_in = nc.dram_tensor("xT_in", [128, 4, NCN], F32, kind="ExternalInput")
    w0_in = nc.dram_tensor("w0_in", [128, 4, 130], F32, kind="ExternalInput")
    w1_in = nc.dram_tensor("w1_in", [128, 1, 1032], F32, kind="ExternalInput")
    w2_in = nc.dram_tensor("w2_in", [128, 4, 328], F32, kind="ExternalInput")
    b0_in = nc.dram_tensor("b0_in", [128, 128], F32, kind="ExternalInput")
    b1_in = nc.dram_tensor("b1_in", [128, 512], F32, kind="ExternalInput")
    b2_in = nc.dram_tensor("b2_in", [128, 64], F32, kind="ExternalInput")
    idx_in = nc.dram_tensor("idx_in", [128, NGRP * EPG // 16], dt.int16, kind="ExternalInput")
    off_in = nc.dram_tensor("off_in", [128, NCOL], BF16, kind="ExternalInput")
    ohT_in = nc.dram_tensor("ohT_in", [GRP, NCOL, 128], dt.float8e4, kind="ExternalInput")

    logits_out = nc.dram_tensor("logits_out", [NCN, C], F32, kind="ExternalOutput")
    prior_out = nc.dram_tensor("prior_out", [NGRP, GRP], F32, kind="ExternalOutput")
    alpha_out = nc.dram_tensor("alpha_out", [128, NCOL, H], F32, kind="ExternalOutput")

    w_ins = [w0_in, w1_in, w2_in]
    b_ins = [b0_in, b1_in, b2_in]

    with tile.TileContext(nc) as tc:
        with (
            tc.tile_pool(name="const", bufs=1) as cp,
            tc.tile_pool(name="state", bufs=1) as st,
            tc.tile_pool(name="dram", bufs=1, space="DRAM") as dp,
        ):
            # ---- constants
            w_t = [cp.tile(list(w.shape), F32, name=f"w{i}_t")
                   for i, w in enumerate(w_ins)]
            b_t = [cp.tile(list(b.shape), F32, name=f"b{i}_t")
                   for i, b in enumerate(b_ins)]
            for s, d in zip(w_ins + b_ins, w_t + b_t):
                nc.sync.dma_start(d[:], s[:])
            idx_sb = cp.tile([128, NGRP * EPG // 16], dt.int16, name="idx_sb")
            nc.sync.dma_start(idx_sb[:], idx_in[:])
            off_sb = cp.tile([128, NCOL], BF16, name="off_sb")
            nc.sync.dma_start(off_sb[:], off_in[:])

            iota_i = cp.tile([128, 128], dt.int32, name="iota_i")
            nc.gpsimd.iota(iota_i[:], pattern=[[1, 128]], base=0, channel_multiplier=0)
            iota_bf = cp.tile([128, 128], BF16, name="iota_bf")
            nc.vector.tensor_copy(iota_bf[:], iota_i[:])
            iotap_i = cp.tile([128, 1], dt.int32, name="iotap_i")
            nc.gpsimd.iota(iotap_i[:], pattern=[[1, 1]], base=0, channel_multiplier=1)
            iotap_f = cp.tile([128, 1], F32, name="iotap_f")
            nc.vector.tensor_copy(iotap_f[:], iotap_i[:])
            iota_f = cp.tile([128, 128], F32, name="iota_f")
            nc.vector.tensor_copy(iota_f[:], iota_i[:])
            zbias = cp.tile([128, 1], F32, name="zbias")
            nc.vector.memset(zbias[:], 0.0)
            ident = cp.tile([128, 128], F32, name="ident")
            nc.vector.tensor_scalar(ident[:], iota_f[:], iotap_f[:], None,
                                    mybir.AluOpType.is_equal)

            # ---- state
            h1T = st.tile([128, NGRP, GRP], F32, name="h1T")
            h2T = st.tile([128, 4, NGRP, GRP], F32, name="h2T")
            er_sb = st.tile([GRP, NGRP, 8], BF16, name="er_sb")
            res2_sb = st.tile([GRP, NGRP, C], F32, name="res2_sb")
            prior_sb = st.tile([GRP, NGRP], F32, name="prior_sb")

            # ---- DRAM scratch
            ag_in = [dp.tile([NCN, L["ROW"]], BF16, name=f"ag{i}")
                     for i, L in enumerate(LAYERS)]
            REPS_ = int(os.environ.get("GAT_REPS", "1"))
            tables_r = [[dp.tile([N, L["ROW"]], BF16, addr_space="Shared",
                                 name=f"table{i}_r{r}")
                         for i, L in enumerate(LAYERS)]
                        for r in range(REPS_)]
            res1_d = dp.tile([NCN, 512], F32, name="res1_d")

            NLAYERS = int(os.environ.get("GAT_LAYERS", "3"))
            REPS = int(os.environ.get("GAT_REPS", "1"))
            DO_AG = os.environ.get("GAT_AG", "1") == "1"
            DO_EDGE = os.environ.get("GAT_EDGE", "1") == "1"
            ESTAGE = int(os.environ.get("GAT_ESTAGE", "9"))
            for rep in range(REPS):
             tables = tables_r[rep]
             for li, L in enumerate(LAYERS[:NLAYERS]):
                Kc, Wf, Wr, HH, Fl, HB = (L["Kc"], L["Wf"], L["Wr"], L["HH"],
                                          L["Fl"], L["HB"])
                ROW, EL32, act = L["ROW"], L["EL32"], L["act"]
                wt = w_t[li]

                # ============================ node phase =====================
                with (
                    tc.tile_pool(name=f"np{li}", bufs=2) as npool,
                    tc.tile_pool(name=f"npp{li}", bufs=2, space="PSUM") as npp,
                ):
                    for g in range(NGRP):
                        # lhsT source
                        if li == 0:
                            hT_t = npool.tile([128, Kc, GRP], F32, name="hT_t")
                            nc.sync.dma_start(hT_t[:], xT_in[:, :, g * GRP:(g + 1) * GRP])
                            def lhs(k):
                                return hT_t[:, k, :]
                        elif li == 1:
                            def lhs(k):
                                return h1T[:, g, :]
                        else:
                            def lhs(k):
                                return h2T[:, k, g, :]

                        TOT = Wf + Wr + 2 * HH
                        if TOT <= 512:
                            # one fused matmul: [feat|res|elr]
                            all_ps = npp.tile([GRP, TOT], F32, name="feat_ps")
                            for k in range(Kc):
                                nc.tensor.matmul(all_ps[:], lhs(k), wt[:, k, 0:TOT],
                                                 start=(k == 0), stop=(k == Kc - 1))
                            feat_ps = all_ps[:, 0:Wf]
                            res_ps = all_ps[:, Wf:Wf + Wr] if Wr else None
                            elr_ps = all_ps[:, Wf + Wr:TOT]
                        else:
                            feat_tp = npp.tile([GRP, Wf], F32, name="feat_ps")
                            elr_tp = npp.tile([GRP, 2 * HH], F32, name="elr_ps")
                            res_tp = (npp.tile([GRP, Wr], F32, name="res_ps")
                                      if Wr else None)
                            for k in range(Kc):
                                nc.tensor.matmul(feat_tp[:], lhs(k), wt[:, k, 0:Wf],
                                                 start=(k == 0), stop=(k == Kc - 1))
                                if Wr:
                                    nc.tensor.matmul(res_tp[:], lhs(k),
                                                     wt[:, k, Wf:Wf + Wr],
                                                     start=(k == 0), stop=(k == Kc - 1))
                                nc.tensor.matmul(elr_tp[:], lhs(k),
                                                 wt[:, k, Wf + Wr:Wf + Wr + 2 * HH],
                                                 start=(k == 0), stop=(k == Kc - 1))
                            feat_ps = feat_tp[:]
                            res_ps = res_tp[:] if Wr else None
                            elr_ps = elr_tp[:]

                        # pack row = [feat|1]*HH + el
                        row_t = npool.tile([GRP, ROW], BF16, name="row_t")
                        nc.vector.memset(row_t[:], 0.0)
                        rhead = row_t[:, 0:HH * HB].rearrange(
                            "p (h b) -> p h b", b=HB)
                        nc.vector.tensor_copy(
                            rhead[:, :, 0:Fl],
                            feat_ps.rearrange("p (h f) -> p h f", f=Fl))
                        nc.vector.memset(rhead[:, :, Fl:Fl + 1], 1.0)
                        nc.vector.tensor_copy(
                            row_t[:, 2 * EL32:2 * EL32 + 2 * HH].bitcast(F32),
                            elr_ps[:, 0:HH])
                        # er hi/lo split (bf16 pair for exact fp32 rebuild)
                        nc.vector.tensor_copy(er_sb[:, g, 0:HH], elr_ps[:, HH:2 * HH])
                        hi_f = npool.tile([GRP, HH], F32, name="hi_f")
                        nc.vector.tensor_copy(hi_f[:], er_sb[:, g, 0:HH])
                        nc.vector.tensor_tensor(er_sb[:, g, HH:2 * HH],
                                                elr_ps[:, HH:2 * HH], hi_f[:],
                                                mybir.AluOpType.subtract)
                        # residual stash
                        if li == 1:
                            res_t = npool.tile([GRP, 512], F32, name="res_t")
                            nc.vector.tensor_copy(res_t[:], res_ps)
                            nc.sync.dma_start(res1_d[g * GRP:(g + 1) * GRP, :], res_t[:])
                        elif li == 2:
                            nc.vector.tensor_copy(res2_sb[:, g, :], res_ps[:, 0:C])
                        nc.sync.dma_start(ag_in[li][g * GRP:(g + 1) * GRP, :], row_t[:])

                # ============================ all-gather =====================
                if DO_AG:
                    nc.gpsimd.collective_compute(
                        "AllGather", mybir.AluOpType.bypass,
                        replica_groups=[list(range(NCORES))],
                        ins=[ag_in[li].opt()], outs=[tables[li].opt()],
                    )
                if not DO_EDGE:
                    continue

                # ============================ edge phase =====================
                with (
                    tc.tile_pool(name=f"eg{li}", bufs=3) as gp,
                    tc.tile_pool(name=f"eo{li}", bufs=2) as op,
                    tc.tile_pool(name=f"es{li}", bufs=2) as sp_pool,
                    tc.tile_pool(name=f"ew{li}", bufs=2) as wp,
                    tc.tile_pool(name=f"epn{li}", bufs=(2 if HH == 1 else 1), space="PSUM") as pn,
                    tc.tile_pool(name=f"epe{li}", bufs=2, space="PSUM") as pe,
                ):
                    if li == 2:
                        exbf_sb = wp.tile([128, NCOL, H], BF16, name="exbf_sb",
                                          bufs=1)
                    for g in range(int(os.environ.get('GAT_NGRP', NGRP))):
                        grow_t = gp.tile([128, NCHUNK, ROW], BF16, name="grow")
                        nc.gpsimd.dma_gather(
                            grow_t[:], tables[li][:],
                            idx_sb[:, g * (EPG // 16):(g + 1) * (EPG // 16)],
                            num_idxs=EPG, num_idxs_reg=EPG, elem_size=ROW,
                            single_packet=False,
                        )
                        grow = grow_t[:]
                        ohT_t = op.tile([GRP, NCHUNK, 128], dt.float8e4, name="ohT_t")
                        nc.sync.dma_start(
                            ohT_t[:], ohT_in[:, g * NCHUNK:(g + 1) * NCHUNK, :])
                        ohT = ohT_t[:]
                        if li == 1:
                            res_rd = op.tile([GRP, 512], F32, name="res_rd")
                            nc.sync.dma_start(res_rd[:], res1_d[g * GRP:(g + 1) * GRP, :])

                        if ESTAGE <= 1:
                            continue
                        # er_dst via one-hot-transpose matmul (hi+lo)
                        er_ps = pe.tile([128, NCHUNK * 8], F32, name="er_ps")
                        for ck in range(NCHUNK):
                            nc.tensor.matmul(er_ps[:, ck * 8:ck * 8 + 2 * HH],
                                             ohT[:, ck, :], er_sb[:, g, 0:2 * HH],
                                             start=True, stop=True)

                        if ESTAGE <= 2:
                            continue
                        # e = lrelu(el_src + er_hi + er_lo); ex = exp(e)
                        erv = er_ps[:].rearrange("p (c x) -> p c x", x=8)
                        elv = grow.bitcast(F32)[:, :, EL32:EL32 + HH]
                        e_f = wp.tile([128, NCHUNK, HH], F32, name="e_f")
                        nc.vector.tensor_tensor(e_f[:], elv, erv[:, :, 0:HH],
                                                mybir.AluOpType.add)
                        nc.vector.tensor_tensor(e_f[:], e_f[:], erv[:, :, HH:2 * HH],
                                                mybir.AluOpType.add)
                        lr_t = wp.tile([128, NCHUNK, HH], F32, name="lr_t")
                        nc.vector.tensor_scalar(lr_t[:], e_f[:], NEG_SLOPE, None,
                                                mybir.AluOpType.mult)
                        nc.vector.tensor_tensor(e_f[:], e_f[:], lr_t[:],
                                                mybir.AluOpType.max)
                        ex_f = wp.tile([128, NCHUNK, HH], F32, name="ex_f")
                        nc.scalar.activation(ex_f[:], e_f[:],
                                             mybir.ActivationFunctionType.Exp,
                                             bias=zbias[:])
                        if li == 2:
                            exbf = exbf_sb[:, g * NCHUNK:(g + 1) * NCHUNK, :]
                        else:
                            exbf_t = wp.tile([128, NCHUNK, HH], BF16, name="exbf_t")
                            exbf = exbf_t[:]
                        nc.vector.tensor_copy(exbf, ex_f[:])

                        if ESTAGE <= 3:
                            continue
                        # forward one-hot scaled by ex
                        oh_f = op.tile([128, NCHUNK, 128], BF16, name="oh_f")
                        nc.vector.tensor_tensor(
                            oh_f[:],
                            iota_bf[:].unsqueeze(1).broadcast_to([128, NCHUNK, 128]),
                            off_sb[:, g * NCHUNK:(g + 1) * NCHUNK]
                                .unsqueeze(2).broadcast_to([128, NCHUNK, 128]),
                            mybir.AluOpType.is_equal)
                        sp_t = sp_pool.tile([128, NCHUNK, HH, 128], BF16, name="sp_t")
                        nc.vector.tensor_tensor(
                            sp_t[:],
                            oh_f[:].unsqueeze(2).broadcast_to([128, NCHUNK, HH, 128]),
                            (exbf if li == 2 else exbf_t[:])
                                .unsqueeze(3).broadcast_to([128, NCHUNK, HH, 128]),
                            mybir.AluOpType.mult)

                        if ESTAGE <= 4:
                            continue
                        # segment-sum matmuls
                        num_ps = [pn.tile([128, HB], F32, name=f"num{h}")
                                  for h in range(HH)]
                        for ck in range(NCHUNK):
                            for h in range(HH):
                                nc.tensor.matmul(
                                    num_ps[h][:], sp_t[:, ck, h, :],
                                    grow[:, ck, h * HB:(h + 1) * HB],
                                    start=(ck == 0), stop=(ck == NCHUNK - 1))

                        if ESTAGE <= 5:
                            continue
                        # ---- eviction
                        recips = []
                        for h in range(HH):
                            den_t = wp.tile([128, 1], F32, name=f"den{h}")
                            nc.vector.tensor_scalar(den_t[:], num_ps[h][:, Fl:Fl + 1],
                                                    1e-9, None, mybir.AluOpType.max)
                            rec_t = wp.tile([128, 1], F32, name=f"rec{h}")
                            nc.vector.reciprocal(rec_t[:], den_t[:])
                            recips.append(rec_t)

                        if li < 2:
                            rst = wp.tile([GRP, HH * Fl], F32, name="rst")
                            rv = rst[:].rearrange("p (h f) -> p h f", f=Fl)
                            for h in range(HH):
                                nc.vector.tensor_scalar(
                                    rv[:, h, :], num_ps[h][:GRP, 0:Fl],
                                    recips[h][:GRP], None, mybir.AluOpType.mult)
                            if li == 1:
                                nc.vector.tensor_tensor(rst[:], rst[:], res_rd[:],
                                                        mybir.AluOpType.add)
                            nc.vector.tensor_tensor(rst[:], rst[:],
                                                    b_t[li][:GRP, 0:HH * Fl],
                                                    mybir.AluOpType.add)
                            # elu = max(x,0) - 1 + exp(min(x,0))
                            m_t = wp.tile([GRP, HH * Fl], F32, name="m_t")
                            nc.vector.tensor_scalar(m_t[:], rst[:], 0.0, None,
                                                    mybir.AluOpType.min)
                            em_t = wp.tile([GRP, HH * Fl], F32, name="em_t")
                            nc.scalar.activation(em_t[:], m_t[:],
                                                 mybir.ActivationFunctionType.Exp,
                                                 bias=zbias[:GRP])
                            nc.vector.tensor_scalar(rst[:], rst[:], 0.0, -1.0,
                                                    mybir.AluOpType.max,
                                                    mybir.AluOpType.add)
                            nc.vector.tensor_tensor(rst[:], rst[:], em_t[:],
                                                    mybir.AluOpType.add)
                            if li == 1:
                                pr_t = wp.tile([GRP, 1], F32, name="pr_t")
                                nc.vector.tensor_reduce(pr_t[:], rst[:],
                                                        mybir.AxisListType.X,
                                                        mybir.AluOpType.add)
                                nc.vector.tensor_scalar(prior_sb[:, g:g + 1], pr_t[:],
                                                        1.0 / 512, None,
                                                        mybir.AluOpType.mult)
                            # transpose h -> hT
                            for fc in range(HH * Fl // 128):
                                tp_ps = pe.tile([128, GRP], F32, name="tp_ps")
                                nc.tensor.transpose(
                                    tp_ps[:], rst[:, fc * 128:(fc + 1) * 128],
                                    ident[:GRP, :GRP])
                                if li == 0:
                                    nc.vector.tensor_copy(h1T[:, g, :], tp_ps[:])
                                else:
                                    nc.vector.tensor_copy(h2T[:, fc, g, :], tp_ps[:])
                        else:
                            # logits = 0.25*sum_h num_h/den_h + res2 + b2m
                            lg = wp.tile([GRP, C], F32, name="lg")
                            nc.vector.tensor_scalar(lg[:], num_ps[0][:GRP, 0:C],
                                                    recips[0][:GRP], 0.25,
                                                    mybir.AluOpType.mult,
                                                    mybir.AluOpType.mult)
                            for h in range(1, HH):
                                t_h = wp.tile([GRP, C], F32, name="t_h")
                                nc.vector.tensor_scalar(t_h[:], num_ps[h][:GRP, 0:C],
                                                        recips[h][:GRP], 0.25,
                                                        mybir.AluOpType.mult,
                                                        mybir.AluOpType.mult)
                                nc.vector.tensor_tensor(lg[:], lg[:], t_h[:],
                                                        mybir.AluOpType.add)
                            nc.vector.tensor_tensor(lg[:], lg[:], res2_sb[:, g, :],
                                                    mybir.AluOpType.add)
                            nc.vector.tensor_tensor(lg[:], lg[:], b_t[2][:GRP, :],
                                                    mybir.AluOpType.add)
                            nc.sync.dma_start(logits_out[g * GRP:(g + 1) * GRP, :], lg[:])

                            # alpha = ex * (1/den)[dst]
                            rec4 = wp.tile([GRP, HH], F32, name="rec4")
                            for h in range(HH):
                                nc.vector.tensor_copy(rec4[:, h:h + 1], recips[h][:GRP])
                            rec_hl = wp.tile([GRP, 8], BF16, name="rec_hl")
                            nc.vector.tensor_copy(rec_hl[:, 0:HH], rec4[:])
                            rhi_f = wp.tile([GRP, HH], F32, name="rhi_f")
                            nc.vector.tensor_copy(rhi_f[:], rec_hl[:, 0:HH])
                            nc.vector.tensor_tensor(rec_hl[:, 4:4 + HH], rec4[:],
                                                    rhi_f[:], mybir.AluOpType.subtract)
                            rd_ps = pe.tile([128, NCHUNK * 8], F32, name="rd_ps")
                            for ck in range(NCHUNK):
                                nc.tensor.matmul(rd_ps[:, ck * 8:ck * 8 + 8],
                                                 ohT[:, ck, :], rec_hl[:],
                                                 start=True, stop=True)
                            rdv = rd_ps[:].rearrange("p (c x) -> p c x", x=8)
                            radd = wp.tile([128, NCHUNK, HH], F32, name="radd")
                            nc.vector.tensor_copy(radd[:], rdv[:, :, 0:HH])
                            nc.vector.tensor_tensor(radd[:], radd[:],
                                                    rdv[:, :, 4:4 + HH],
                                                    mybir.AluOpType.add)
                            al_t = wp.tile([128, NCHUNK, HH], F32, name="al_t")
                            nc.vector.tensor_tensor(al_t[:], radd[:], exbf,
                                                    mybir.AluOpType.mult)
                            nc.sync.dma_start(
                                alpha_out[:, g * NCHUNK:(g + 1) * NCHUNK, :], al_t[:])

            # prior: [GRP, NGRP] -> transpose -> [NGRP, GRP] -> DMA
            if NLAYERS >= 2 and DO_EDGE:
              with tc.tile_pool(name="fin", bufs=1) as fp, \
                 tc.tile_pool(name="finp", bufs=1, space="PSUM") as fpp:
                pr_ps = fpp.tile([NGRP, GRP], F32, name="pr_ps")
                nc.tensor.transpose(pr_ps[:], prior_sb[:], ident[:GRP, :GRP])
                pr_sb = fp.tile([NGRP, GRP], F32, name="pr_sb")
                nc.vector.tensor_copy(pr_sb[:], pr_ps[:])
                nc.sync.dma_start(prior_out[:], pr_sb[:])


    nc.compile()
    return nc


_KERNEL_CACHE = {}


def kernel(x, src, dst, W0, al0, ar0, b0, W1, al1, ar1, b1, resW1,
           W2, al2, ar2, b2, resW2):
    x = np.asarray(x, np.float32)
    graph = _prep_graph(src, dst)
    w0, w1, w2, b0r, b1r, b2r = _prep_weights(
        np.asarray(W0), np.asarray(al0), np.asarray(ar0), np.asarray(b0),
        np.asarray(W1), np.asarray(al1), np.asarray(ar1), np.asarray(b1),
        np.asarray(resW1),
        np.asarray(W2), np.asarray(al2), np.asarray(ar2), np.asarray(b2),
        np.asarray(resW2))

    if "nc" not in _KERNEL_CACHE:
        _KERNEL_CACHE["nc"] = _build_kernel()
    nc = _KERNEL_CACHE["nc"]

    in_maps = []
    for c in range(NCORES):
        xc = x[c * NCN:(c + 1) * NCN]          # [2500, 512]
        xT = np.ascontiguousarray(
            xc.T.reshape(4, 128, NCN).transpose(1, 0, 2))
        g = graph[c]
        in_maps.append({
            "xT_in": xT, "w0_in": w0, "w1_in": w1, "w2_in": w2,
            "b0_in": b0r, "b1_in": b1r, "b2_in": b2r,
            "idx_in": g["idx_in"], "off_in": g["off_in"], "ohT_in": g["ohT_in"],
        })

    global _LAST_IN_MAPS
    _LAST_IN_MAPS = in_maps
    res = run_bass_kernel_spmd(nc, in_maps, core_ids=list(range(NCORES)))

    logits = np.concatenate([res.results[c]["logits_out"] for c in range(NCORES)], 0)
    prior = np.concatenate([res.results[c]["prior_out"].reshape(-1) for c in range(NCORES)], 0)
    att = np.zeros((E, H, 1), np.float32)
    for c in range(NCORES):
        em = graph[c]["em"]                    # [128, NCOL]
        al = res.results[c]["alpha_out"]       # [128, NCOL, H]
        valid = em >= 0
        att[em[valid], :, 0] = al[valid, :]
    return logits, prior, att
